# revision 7
# baseline (speedup 1.0000x reference)
"""Trainium2 Bass kernel for nn_AttentionBlock (GroupNorm32 + QKVAttentionLegacy + proj).

Sharding: 8 cores = 4 batch x 2 L-halves. Each core computes the full block for
one batch element restricted to a 2048-query half; keys/values span all 4096
positions. Odd-half cores receive x with the two L-halves swapped (attention is
permutation-invariant over key positions), so one SPMD program serves all cores
with static slicing and zero collectives.

v2 pipeline (per core):
  - GroupNorm folded into qkv weights (as v1), fp32 preamble.
  - qkv projections fp32r; k kept bias-free (bias cancels in softmax over s);
    v^T written as fp8 s-block pairs [128s, 2, head, 65] with a ones column.
  - scores[s,t] per (head, tau=512q, s-block): one K=64 matmul, fp32r.
  - softmax exp split across 3 engines by s-block pair: ACT does exact
    exp->fp8e4; Pool/DVE do one-op Schraudolph exp (x*A+B -> uint8 == fp8
    bits, round-to-nearest convert). exp(0.125*s - 2): shift cancels in the
    normalization.
  - av^T[t,c] per (head, t-128-block): 16 fp8 DoubleRow matmuls (contract 2
    s-blocks each), out free = 65 (64 ch + denominator column).
  - normalize on DVE (per-partition reciprocal), PE-transpose a^T -> a (bf16),
    proj with K=128 bf16 matmuls, fused (+bias)+residual epilogue.
"""

import sys

import numpy as np

for _p in ("/opt/trn_rl_repo",):
    if _p not in sys.path:
        sys.path.insert(0, _p)

NUM_HEADS = 4
C = 256
L = 4096
T = 2048  # per-core query half
CH = 64
GROUPS = 32
EPS = 1e-5

# Schraudolph exp -> fp8e4 bits: bits = rint(s*A8 + B8) with logit scale 1/8
# and shift -2 folded in; c8 centers the sawtooth error band.
_C8 = -0.344
A8 = 0.125 * 8.0 / np.log(2.0)
B8 = 56.0 - 2.0 * (8.0 / np.log(2.0)) + _C8

_CACHE = {}


def _build_bass(loop_n=None, act_blocks=18):
    import ml_dtypes
    import concourse.tile as tile
    from concourse import bacc, mybir

    f32 = mybir.dt.float32
    bf16 = mybir.dt.bfloat16
    u8 = mybir.dt.uint8
    fp8 = mybir.dt.float8e4
    AF = mybir.ActivationFunctionType
    OP = mybir.AluOpType
    PM = mybir.MatmulPerfMode

    nc = bacc.Bacc()
    f32r = mybir.dt.float32r
    R = lambda ap: ap.bitcast(f32r)  # FP22 matmul path: 1 cyc/row vs fp32's 4

    xp_d = nc.dram_tensor("xp", [C, L], f32, kind="ExternalInput")
    wqkvT_d = nc.dram_tensor("wqkvT", [C, 3 * C], f32, kind="ExternalInput")
    wprojT_d = nc.dram_tensor("wprojT", [C, C], f32, kind="ExternalInput")
    qkvb_d = nc.dram_tensor("qkvb", [3 * C], f32, kind="ExternalInput")
    gnw_d = nc.dram_tensor("gnw", [C], f32, kind="ExternalInput")
    gnb_d = nc.dram_tensor("gnb", [C], f32, kind="ExternalInput")
    projb_d = nc.dram_tensor("projb", [C], f32, kind="ExternalInput")
    out_d = nc.dram_tensor("out", [C, T], f32, kind="ExternalOutput")

    # group indicator matrices for partition-dim group reductions via PE
    ig_np = np.zeros((C, GROUPS), np.float32)
    ig_np[np.arange(C), np.arange(C) // 8] = 1.0
    ig_d = nc.inline_tensor(ig_np, "ig")
    igT_d = nc.inline_tensor(np.ascontiguousarray(ig_np.T), "igT")
    ident_d = nc.inline_tensor(np.eye(128, dtype=ml_dtypes.bfloat16), "ident")

    # engine schedule for the 32 s-blocks: GPSIMD cannot touch PSUM, so exp
    # splits across ACT (exact exp->fp8) and DVE (Schraudolph bits) only;
    # interleave so consecutive score tiles drain to different engines
    sched = []
    acc = {"A": 0.0, "D": 0.0}
    counts = {"A": act_blocks, "D": 32 - act_blocks}
    for _ in range(32):
        for e in ("A", "D"):
            acc[e] += counts[e] / 32.0
        pick = max(acc, key=lambda e: acc[e])
        acc[pick] -= 1.0
        sched.append(pick)

    with tile.TileContext(nc) as tc:
        from contextlib import ExitStack, nullcontext

        ctx = ExitStack()
        with ctx:
            loop = tc.For_i(0, loop_n, 1) if loop_n else nullcontext()
            ctx.enter_context(loop)
            singles = ctx.enter_context(tc.tile_pool(name="singles", bufs=1))
            ew_pool = ctx.enter_context(tc.tile_pool(name="ew", bufs=3))
            atmp = ctx.enter_context(tc.tile_pool(name="atmp", bufs=12))
            rbp = ctx.enter_context(tc.tile_pool(name="rbp", bufs=8))
            outp = ctx.enter_context(tc.tile_pool(name="outp", bufs=3))

            # ---------------- load inputs ----------------
            xs = singles.tile([128, 2, L], f32, tag="xs")
            for t in range(2):
                for j in range(2):
                    nc.sync.dma_start(
                        out=R(xs[:, t, j * 2048 : (j + 1) * 2048]),
                        in_=xp_d[t * 128 : (t + 1) * 128, j * 2048 : (j + 1) * 2048].bitcast(f32r),
                    )
            qkvb = singles.tile([128, 6], f32, tag="qkvb")
            nc.sync.dma_start(out=qkvb, in_=qkvb_d[:].rearrange("(m p) -> p m", p=128))
            gnw = singles.tile([128, 2], f32, tag="gnw")
            nc.sync.dma_start(out=gnw, in_=gnw_d[:].rearrange("(t p) -> p t", p=128))
            gnb = singles.tile([128, 2], f32, tag="gnb")
            nc.sync.dma_start(out=gnb, in_=gnb_d[:].rearrange("(t p) -> p t", p=128))
            projb = singles.tile([128, 2], f32, tag="projb")
            nc.sync.dma_start(out=projb, in_=projb_d[:].rearrange("(t p) -> p t", p=128))
            ig = singles.tile([128, 2, GROUPS], f32, tag="ig")
            for t in range(2):
                nc.sync.dma_start(out=ig[:, t, :], in_=ig_d[t * 128 : (t + 1) * 128, :])
            igT = singles.tile([GROUPS, C], f32, tag="igT")
            nc.sync.dma_start(out=igT, in_=igT_d[:, :])
            ident = singles.tile([128, 128], bf16, tag="ident")
            nc.sync.dma_start(out=ident, in_=ident_d[:, :])
            nbias = singles.tile([128, 1], f32, tag="nbias")
            nc.vector.memset(nbias, -2.0)

            pw_bf = singles.tile([128, 2, C], bf16, tag="pw_bf")

            # ---------------- GroupNorm stats -> A, B ----------------
            with tc.tile_pool(name="gn_ps", bufs=1, space="PSUM") as gn_ps, \
                 tc.tile_pool(name="gn_wk", bufs=1) as gn_wk:
                wt = gn_wk.tile([128, 2, 3 * C], f32, tag="wt")
                for t in range(2):
                    nc.sync.dma_start(out=wt[:, t, :], in_=wqkvT_d[t * 128 : (t + 1) * 128, :])
                pt = gn_wk.tile([128, 2, C], f32, tag="pt")
                for t in range(2):
                    nc.sync.dma_start(out=pt[:, t, :], in_=wprojT_d[t * 128 : (t + 1) * 128, :])
                for t in range(2):
                    nc.vector.tensor_copy(out=pw_bf[:, t, :], in_=pt[:, t, :])
                stats = gn_wk.tile([128, 2, 8, 6], f32, tag="stats")
                for t in range(2):
                    for j in range(8):
                        nc.vector.bn_stats(
                            out=stats[:, t, j, :],
                            in_=xs[:, t, j * 512 : (j + 1) * 512],
                        )
                mv = gn_wk.tile([128, 2, 2], f32, tag="mv")
                for t in range(2):
                    nc.vector.bn_aggr(out=mv[:, t, :], in_=stats[:, t, :, :])
                # per-channel {mean, var, mean^2}
                pcs = gn_wk.tile([128, 2, 3], f32, tag="pcs")
                for t in range(2):
                    nc.vector.tensor_copy(out=pcs[:, t, 0:2], in_=mv[:, t, :])
                    nc.vector.tensor_mul(
                        out=pcs[:, t, 2:3], in0=mv[:, t, 0:1], in1=mv[:, t, 0:1]
                    )
                gsum = gn_ps.tile([GROUPS, 3], f32, tag="gsum")
                for t in range(2):
                    nc.tensor.matmul(
                        gsum, ig[:, t, :], pcs[:, t, :], start=(t == 0), stop=(t == 1)
                    )
                gstats = gn_wk.tile([GROUPS, 3], f32, tag="gstats")
                nc.vector.tensor_scalar_mul(out=gstats, in0=gsum, scalar1=0.125)
                varg = gn_wk.tile([GROUPS, 1], f32, tag="varg")
                nc.vector.tensor_add(out=varg, in0=gstats[:, 1:2], in1=gstats[:, 2:3])
                mg2 = gn_wk.tile([GROUPS, 1], f32, tag="mg2")
                nc.vector.tensor_mul(out=mg2, in0=gstats[:, 0:1], in1=gstats[:, 0:1])
                nc.vector.tensor_tensor(
                    out=varg, in0=varg, in1=mg2, op=OP.subtract
                )
                # rstd = exp(-0.5 * ln(var + eps)) - stays in the exp table set
                eps_t = gn_wk.tile([GROUPS, 1], f32, tag="eps_t")
                nc.vector.memset(eps_t, EPS)
                lnv = gn_wk.tile([GROUPS, 1], f32, tag="lnv")
                nc.scalar.activation(out=lnv, in_=varg, func=AF.Ln, bias=eps_t)
                stats2 = gn_wk.tile([GROUPS, 2], f32, tag="stats2")
                nc.vector.tensor_copy(out=stats2[:, 0:1], in_=gstats[:, 0:1])
                nc.scalar.activation(
                    out=stats2[:, 1:2], in_=lnv, func=AF.Exp, scale=-0.5
                )
                cstat = gn_ps.tile([128, 2, 2], f32, tag="cstat")
                for t in range(2):
                    nc.tensor.matmul(
                        cstat[:, t, :],
                        igT[:, t * 128 : (t + 1) * 128],
                        stats2,
                        start=True,
                        stop=True,
                    )
                # A = rstd_c * gn_w ; B = gn_b - mean_c * A
                ab = singles.tile([128, 2, 2], f32, tag="ab")  # [..0]=A [..1]=B
                for t in range(2):
                    nc.vector.tensor_mul(
                        out=ab[:, t, 0:1], in0=cstat[:, t, 1:2], in1=gnw[:, t : t + 1]
                    )
                    nc.vector.tensor_mul(
                        out=ab[:, t, 1:2], in0=cstat[:, t, 0:1], in1=ab[:, t, 0:1]
                    )
                    nc.vector.tensor_tensor(
                        out=ab[:, t, 1:2],
                        in0=gnb[:, t : t + 1],
                        in1=ab[:, t, 1:2],
                        op=OP.subtract,
                    )
                # scaled qkv weights
                wts = singles.tile([128, 2, 3 * C], f32, tag="wts")
                for t in range(2):
                    nc.vector.tensor_scalar_mul(
                        out=R(wts[:, t, :]), in0=wt[:, t, :], scalar1=ab[:, t, 0:1]
                    )
                # bias_full = W @ B + qkv_b   (unscaled W)
                bf_ps = gn_ps.tile([128, 6], f32, tag="bf_ps")
                for m in range(6):
                    for t in range(2):
                        nc.tensor.matmul(
                            bf_ps[:, m : m + 1],
                            wt[:, t, m * 128 : (m + 1) * 128],
                            ab[:, t, 1:2],
                            start=(t == 0),
                            stop=(t == 1),
                        )
                biasf = singles.tile([128, 6], f32, tag="biasf")
                nc.vector.tensor_add(out=biasf, in0=bf_ps, in1=qkvb)
                # proj bias' = proj_b + P @ gamma, gamma = biasf v-part
                pb_ps = gn_ps.tile([128, 2], f32, tag="pb_ps")
                for m in range(2):
                    for t in range(2):
                        nc.tensor.matmul(
                            pb_ps[:, m : m + 1],
                            pt[:, t, m * 128 : (m + 1) * 128],
                            biasf[:, 4 + t : 5 + t],
                            start=(t == 0),
                            stop=(t == 1),
                        )
                pbf = singles.tile([128, 2], f32, tag="pbf")
                nc.vector.tensor_add(out=pbf, in0=pb_ps, in1=projb)

            # ---------------- qkv projections ----------------
            q_sb = singles.tile([128, 2, T], f32, tag="q_sb")
            k_sb = singles.tile([128, 2, L], f32, tag="k_sb")
            # v^T as fp8 s-block pairs + ones column for the denominator
            vt8 = singles.tile([128, 16, 2, NUM_HEADS, 65], fp8, tag="vt8")
            nc.vector.memset(vt8[:, :, :, :, 64:65], 1.0)

            with tc.tile_pool(name="qkv_ps", bufs=3, space="PSUM") as qkv_ps:
                for m in range(2):
                    for n in range(8):  # k: full L, no bias (cancels in softmax)
                        pp = qkv_ps.tile([128, 512], f32, tag="pp")
                        for t in range(2):
                            nc.tensor.matmul(
                                pp,
                                R(wts[:, t, 256 + m * 128 : 256 + (m + 1) * 128]),
                                R(xs[:, t, n * 512 : (n + 1) * 512]),
                                start=(t == 0),
                                stop=(t == 1),
                            )
                        nc.scalar.copy(
                            out=R(k_sb[:, m, n * 512 : (n + 1) * 512]), in_=pp
                        )
                    for n in range(4):  # q: this core's half only, +bias
                        pp = qkv_ps.tile([128, 512], f32, tag="pp")
                        for t in range(2):
                            nc.tensor.matmul(
                                pp,
                                R(wts[:, t, m * 128 : (m + 1) * 128]),
                                R(xs[:, t, n * 512 : (n + 1) * 512]),
                                start=(t == 0),
                                stop=(t == 1),
                            )
                        nc.vector.tensor_scalar_add(
                            out=R(q_sb[:, m, n * 512 : (n + 1) * 512]),
                            in0=pp,
                            scalar1=biasf[:, m : m + 1],
                        )
                for sl in range(32):  # v^T all heads, no bias (folded into proj)
                    pp = qkv_ps.tile([128, 512], f32, tag="pp")
                    vv = pp[:, 0:256]
                    for t in range(2):
                        nc.tensor.matmul(
                            vv,
                            R(xs[:, t, sl * 128 : (sl + 1) * 128]),
                            R(wts[:, t, 512:768]),
                            start=(t == 0),
                            stop=(t == 1),
                        )
                    nc.scalar.copy(
                        out=vt8[:, sl // 2, sl % 2, :, 0:64], in_=vv
                    )

            # ---------------- attention ----------------
            a_sb = singles.tile([128, 2, T], bf16, tag="a_sb")

            with tc.tile_pool(name="at_ps", bufs=1, space="PSUM") as at_ps:
                sc_all = at_ps.tile([128, 4, 512], f32, tag="sc_all")
                av_all = at_ps.tile([128, 2, 4, 128], f32, tag="av_all")
                tr_all = at_ps.tile([128, 8, 128], bf16, tag="tr_all")

                units = [(tau, h) for tau in range(4) for h in range(4)]
                at_tiles = {}  # (ui, tsub) -> a^T bf16 sbuf tile
                tr_slots = {}  # (tau, cb) live while h-pair in flight

                def stage_scores_exp(ui):
                    tau, h = units[ui]
                    pair, lo = h // 2, 64 * (h % 2)
                    t0 = tau * 512
                    ew = ew_pool.tile([128, 16, 2, 512], fp8, tag="ew")
                    for sb in range(32):
                        p, j = sb // 2, sb % 2
                        st = sc_all[:, sb % 4, :]
                        nc.tensor.matmul(
                            st,
                            R(k_sb[lo : lo + 64, pair, sb * 128 : (sb + 1) * 128]),
                            R(q_sb[lo : lo + 64, pair, t0 : t0 + 512]),
                            start=True,
                            stop=True,
                            tile_position=(lo, 0),
                        )
                        e = sched[sb]
                        if e == "A":
                            nc.scalar.activation(
                                out=ew[:, p, j, :],
                                in_=st,
                                func=AF.Exp,
                                scale=0.125,
                                bias=nbias,
                            )
                        else:
                            nc.vector.tensor_scalar(
                                out=ew[:, p, j, :].bitcast(u8),
                                in0=st,
                                scalar1=float(A8),
                                scalar2=float(B8),
                                op0=OP.mult,
                                op1=OP.add,
                            )
                    return ew

                def stage_av_norm(ui, ew):
                    tau, h = units[ui]
                    for tsub in range(4):
                        av = av_all[:, ui % 2, tsub, 0:65]
                        for p in range(16):
                            nc.tensor.matmul(
                                av,
                                ew[:, p, :, tsub * 128 : (tsub + 1) * 128],
                                vt8[:, p, :, h, :],
                                start=(p == 0),
                                stop=(p == 15),
                                perf_mode=PM.DoubleRow,
                            )
                    for tsub in range(4):
                        rb = rbp.tile([128, 1], f32, tag="rb")
                        nc.vector.reciprocal_approx_fast(
                            out=rb, in_=av_all[:, ui % 2, tsub, 64:65]
                        )
                        at = atmp.tile([128, 64], bf16, tag="at")
                        nc.vector.tensor_scalar_mul(
                            out=at, in0=av_all[:, ui % 2, tsub, 0:64], scalar1=rb
                        )
                        at_tiles[(ui, tsub)] = at

                def stage_tr_copy(ui):
                    tau, h = units[ui]
                    cb, odd = h // 2, h % 2
                    for tsub in range(4):
                        slot = cb * 4 + tsub
                        nc.tensor.transpose(
                            tr_all[64 * odd : 64 * odd + 64, slot, :],
                            at_tiles.pop((ui, tsub)),
                            ident,
                        )
                        if odd:
                            nc.vector.tensor_copy(
                                out=a_sb[:, cb, tau * 512 + tsub * 128 : tau * 512 + (tsub + 1) * 128],
                                in_=tr_all[:, slot, :],
                            )

                ew_live = {}
                for i in range(len(units) + 2):
                    if i < len(units):
                        ew_live[i] = stage_scores_exp(i)
                    if 1 <= i <= len(units):
                        stage_av_norm(i - 1, ew_live.pop(i - 1))
                    if 2 <= i <= len(units) + 1:
                        stage_tr_copy(i - 2)

            # ---------------- proj + residual ----------------
            with tc.tile_pool(name="pj_ps", bufs=3, space="PSUM") as pj_ps:
                for m in range(2):
                    for n in range(4):
                        pp = pj_ps.tile([128, 512], f32, tag="pj")
                        for cb in range(2):
                            nc.tensor.matmul(
                                pp,
                                pw_bf[:, cb, m * 128 : (m + 1) * 128],
                                a_sb[:, cb, n * 512 : (n + 1) * 512],
                                start=(cb == 0),
                                stop=(cb == 1),
                            )
                        ot = outp.tile([128, 512], f32, tag="ot")
                        nc.vector.scalar_tensor_tensor(
                            out=ot,
                            in0=pp,
                            scalar=pbf[:, m : m + 1],
                            in1=xs[:, m, n * 512 : (n + 1) * 512],
                            op0=OP.add,
                            op1=OP.add,
                        )
                        nc.sync.dma_start(
                            out=out_d[m * 128 : (m + 1) * 128, n * 512 : (n + 1) * 512],
                            in_=ot,
                        )

    nc.finalize()
    return nc


def _get_nc():
    if "nc" not in _CACHE:
        _CACHE["nc"] = _build_bass()
    return _CACHE["nc"]


def _prepare_in_maps(x, gn_w, gn_b, qkv_w, qkv_b, proj_w, proj_b):
    x = np.asarray(x, np.float32)
    gn_w = np.asarray(gn_w, np.float32)
    gn_b = np.asarray(gn_b, np.float32)
    qkv_w = np.asarray(qkv_w, np.float32)
    qkv_b = np.asarray(qkv_b, np.float32)
    proj_w = np.asarray(proj_w, np.float32)
    proj_b = np.asarray(proj_b, np.float32)

    B, Cx, H, W = x.shape
    xf = x.reshape(B, Cx, H * W)

    # QKVAttentionLegacy: head h owns qkv rows [h*192, (h+1)*192) as q/k/v
    # blocks of 64. Permute to [q by head | k by head | v by head].
    perm = np.concatenate(
        [
            np.arange(h * 192 + j * 64, h * 192 + (j + 1) * 64)
            for j in range(3)
            for h in range(NUM_HEADS)
        ]
    )
    wqkvT = np.ascontiguousarray(qkv_w[perm].T)
    qkvb_p = np.ascontiguousarray(qkv_b[perm])
    wprojT = np.ascontiguousarray(proj_w.T)

    shared = {
        "wqkvT": wqkvT,
        "wprojT": wprojT,
        "qkvb": qkvb_p,
        "gnw": gn_w,
        "gnb": gn_b,
        "projb": proj_b,
    }
    in_maps = []
    for core in range(8):
        b, half = core // 2, core % 2
        if half == 0:
            xp = xf[b]
        else:
            xp = np.concatenate([xf[b][:, T:], xf[b][:, :T]], axis=1)
        in_maps.append({"xp": np.ascontiguousarray(xp), **shared})

    return in_maps, (B, Cx, H, W)


def _assemble(results, shape):
    B, Cx, H, W = shape
    out = np.empty((B, Cx, H * W), np.float32)
    for core in range(8):
        b, half = core // 2, core % 2
        out[b][:, half * T : (half + 1) * T] = results[core]["out"]
    return out.reshape(B, Cx, H, W)


def kernel(x, gn_w, gn_b, qkv_w, qkv_b, proj_w, proj_b):
    from concourse.bass_utils import run_bass_kernel_spmd

    in_maps, shape = _prepare_in_maps(x, gn_w, gn_b, qkv_w, qkv_b, proj_w, proj_b)
    nc = _get_nc()
    res = run_bass_kernel_spmd(nc, in_maps, core_ids=list(range(8)))
    _CACHE["last_results"] = res
    return _assemble(res.results, shape)
